# revision 33
# baseline (speedup 1.0000x reference)
# Trainium2 Bass kernel for nn_DCA_78889959293548 (dual cross/spatial attention
# over 4 feature scales). Data-parallel over the batch dim across 8 NeuronCores.
#
# Math (per batch element, reference semantics):
#   channel branch: xn_i = LN(x_i); xc = concat(xn_i) [N,S]
#     q=xn_i*wq_i, k=xc*wk_i, v=xc*wv_i (heads=1)
#     att = softmax_e(q^T k / sqrt(S));  x_i += (att @ v^T)^T * wp_i
#   spatial branch: xn_i = LN(x_i); xc = concat [N,S]
#     q=xc*wq_i, k=xc*wk_i, v=xn_i*wv_i (heads=4, head dim 480)
#     att = softmax_m(q k^T / sqrt(480)); x_i += (att @ v_h) * wp_i
#
# Kernel layout choices:
#  - channel att computed transposed: expatt_T[e,d] = exp(wk_i[e]*G[e,d]) where
#    G = xc^T @ (xc.wq) via PE; wk fold via per-partition ACT scale.
#    AV as out_T[n,d] = sum_e v[e,n]^T-as-lhsT @ expatt_T; a ones column in the
#    lhsT yields the softmax normalizer Z[d] for free; wp/Z applied via a K=1
#    PE broadcast + reciprocal.
#  - spatial att: channel-major xnsT (via DMA-xbar transposes of the LN output,
#    zero-padded to 512-aligned heads), u=wq.wk folded into one side,
#    exp^T as lhsT for AV, ones column in v gives Z[n] per-partition.
import math
import numpy as np
import ml_dtypes

import concourse.bass as bass
import concourse.mybir as mybir
import concourse.tile as tile
from concourse import bacc
from concourse.bass_utils import run_bass_kernel_spmd
from concourse.mybir import AluOpType, ActivationFunctionType as AFT

BF = mybir.dt.bfloat16
F32 = mybir.dt.float32
NPBF = ml_dtypes.bfloat16

NCORES = 8
B, N, EPS = 128, 192, 1e-6
FEATS = [128, 256, 512, 1024]
OFFS = [0, 128, 384, 896]
S = 1920
EC = 15                     # 128-wide chunks of S
NT = [(0, 128), (128, 64)]  # token tiles (row0, nrows)
# channel AV passes: per pass, scales with their psum placement
# (scale, [(psum_slot, col0_in_slot, width, global_d0), ...]) plus the
# (global_d0, width) per psum slot for the epilogue.
CH_PASSES = [
    dict(scales=[(0, [(0, 0, 128, 0)]), (1, [(0, 128, 256, 128)]), (2, [(1, 0, 512, 384)])],
         slots=[(0, 384), (384, 512)]),
    dict(scales=[(3, [(0, 0, 512, 896), (1, 0, 512, 1408)])],
         slots=[(896, 512), (1408, 512)]),
]


def _bld_params(params):
    """Host-side folding of the tiny per-channel parameters."""
    ca, sa = params["c_att"], params["s_att"]
    cln, sln = params["c_ln"], params["s_ln"]
    for i in range(4):
        for att in (ca[i], sa[i]):
            for k in ("q", "k", "v", "p"):
                assert not np.any(np.asarray(att[k]["b"])), "nonzero dw bias unsupported"
        for lnp in (cln[i], sln[i]):
            assert np.all(np.asarray(lnp["g"]) == 1.0), "LN gain != 1 unsupported"
            assert not np.any(np.asarray(lnp["b"])), "LN bias != 0 unsupported"
    cat = lambda key, grp: np.concatenate([np.asarray(grp[i][key]["w"], np.float32) for i in range(4)])
    wq = cat("q", ca)                                   # [S]
    wk = np.stack([np.asarray(ca[i]["k"]["w"], np.float32) * S ** -0.5 for i in range(4)])  # [4,S]
    wv = np.stack([np.asarray(ca[i]["v"]["w"], np.float32) for i in range(4)])              # [4,S]
    invwp = 1.0 / cat("p", ca)                          # [S]
    u = np.stack([np.asarray(sa[i]["q"]["w"], np.float32) * np.asarray(sa[i]["k"]["w"], np.float32) * 480 ** -0.5
                  for i in range(4)])                   # [4,S]
    upad = np.zeros((4, 2048), np.float32)
    for h in range(4):
        upad[:, 512 * h:512 * h + 480] = u[:, 480 * h:480 * h + 480]
    wvp = np.concatenate([np.asarray(sa[i]["v"]["w"], np.float32) * np.asarray(sa[i]["p"]["w"], np.float32)
                          for i in range(4)])           # [S]
    return {
        "wq_row": wq.astype(NPBF),                                  # [S] bf16, row-bcast
        "wk_col": np.ascontiguousarray(wk.reshape(4, EC, 128).transpose(0, 2, 1)),   # [4,128,15] f32
        "wv_col": np.ascontiguousarray(wv.reshape(4, EC, 128).transpose(0, 2, 1)),   # [4,128,15] f32
        "invwp": invwp.astype(NPBF),                                # [S] bf16
        "u_col": np.ascontiguousarray(upad.reshape(4, 16, 128).transpose(0, 2, 1)),  # [4,128,16] f32
        "wvp_row": wvp.astype(NPBF),                                # [S] bf16, row-bcast
    }


def _layernorm4(nc, pln, eps_col, x_tile, h, dst_tile):
    """LN of all 4 scales of one token tile: stats per scale, then a single
    batched Sqrt+reciprocal (fewer ACT calls -> fewer exp/sqrt table swaps)."""
    mv8 = pln.tile([128, 4, 2], F32, tag="lnmv", name="mv8")
    for i in range(4):
        o, c = OFFS[i], FEATS[i]
        fmax = math.gcd(512, c)
        nsub = c // fmax
        stats = pln.tile([128, 2, 6], F32, tag="lnstats", name="stats")
        for s in range(nsub):
            nc.vector.bn_stats(out=stats[:h, s, :], in_=x_tile[:h, o + s * fmax:o + (s + 1) * fmax])
        nc.vector.bn_aggr(out=mv8[:h, i, :], in_=stats[:h, :nsub, :])
    sd4 = pln.tile([128, 4], F32, tag="lnsd", name="sd4")
    nc.scalar.activation(out=sd4[:h], in_=mv8[:h, :, 1:2].rearrange("p a o -> p (a o)"),
                         func=AFT.Sqrt, bias=eps_col[:h], scale=1.0)
    nc.vector.reciprocal(out=sd4[:h], in_=sd4[:h])
    for i in range(4):
        o, c = OFFS[i], FEATS[i]
        nc.vector.tensor_scalar(out=dst_tile[:h, o:o + c], in0=x_tile[:h, o:o + c],
                                scalar1=mv8[:h, i, 0:1], scalar2=sd4[:h, i:i + 1],
                                op0=AluOpType.subtract, op1=AluOpType.mult)


def build(bl=B // NCORES):
    nc = bacc.Bacc("TRN2", target_bir_lowering=False, debug=False, num_devices=NCORES)
    xin = [nc.dram_tensor(f"x{i}", [bl, N, FEATS[i]], F32, kind="ExternalInput") for i in range(4)]
    yout = [nc.dram_tensor(f"y{i}", [bl, N, FEATS[i]], F32, kind="ExternalOutput") for i in range(4)]
    d_wq = nc.dram_tensor("wq_row", [S], BF, kind="ExternalInput")
    d_wk = nc.dram_tensor("wk_col", [4, 128, EC], F32, kind="ExternalInput")
    d_wv = nc.dram_tensor("wv_col", [4, 128, EC], F32, kind="ExternalInput")
    d_iwp = nc.dram_tensor("invwp", [S], BF, kind="ExternalInput")
    d_u = nc.dram_tensor("u_col", [4, 128, 16], F32, kind="ExternalInput")
    d_wvp = nc.dram_tensor("wvp_row", [S], BF, kind="ExternalInput")

    from contextlib import ExitStack
    with tile.TileContext(nc) as tc:
        with ExitStack() as ctx:
            pool_specs = dict(pcon=1, px=2, pbf=1, pT=1, pv=2, pe=1, pu=1, pse=3,
                              pvs=3, pz=1, prb=2, ptmp=2, prz=4, pln=4)
            pools = {k: ctx.enter_context(tc.tile_pool(name=k, bufs=v))
                     for k, v in pool_specs.items()}
            pools["ps"] = ctx.enter_context(tc.tile_pool(name="ps", bufs=8, space="PSUM"))
            (pcon, px, pbf, pT, pv, pe, pu, pse, pvs, pz, prb, ptmp, prz, pln, ps) = (
                pools[k] for k in ("pcon", "px", "pbf", "pT", "pv", "pe", "pu", "pse",
                                   "pvs", "pz", "prb", "ptmp", "prz", "pln", "ps"))
            # ---- constants ----
            def row_bcast(d):  # [S] dram vector -> [[0,128],[1,S]] partition-bcast read
                a = d.ap()
                return bass.AP(tensor=a.tensor, offset=a.offset, ap=[[0, 128], [1, S]])

            wq_row = pcon.tile([128, S], BF)
            nc.sync.dma_start(out=wq_row, in_=row_bcast(d_wq))
            wvp_row = pcon.tile([128, S], BF)
            nc.sync.dma_start(out=wvp_row, in_=row_bcast(d_wvp))
            wk_col = pcon.tile([128, 4, EC], F32)
            nc.sync.dma_start(out=wk_col, in_=d_wk.ap().rearrange("i p a -> p i a"))
            wv_col = pcon.tile([128, 4, EC], F32)
            nc.sync.dma_start(out=wv_col, in_=d_wv.ap().rearrange("i p a -> p i a"))
            u_col = pcon.tile([128, 4, 16], F32)
            nc.sync.dma_start(out=u_col, in_=d_u.ap().rearrange("i p a -> p i a"))
            invwp = pcon.tile([1, S], BF)
            nc.sync.dma_start(out=invwp, in_=d_iwp.ap().rearrange("(o a) -> o a", o=1))
            ones_k1 = pcon.tile([1, 128], BF)
            nc.vector.memset(ones_k1, 1.0)
            eps_col = pcon.tile([128, 1], F32)
            nc.vector.memset(eps_col, EPS)

            def front(b):
                """loads + channel LN + q-scaled copy + channel-major transpose"""
                st = dict(xf=[], xc=[], xnq=[])
                for t, (r0, h) in enumerate(NT):
                    xt = px.tile([128, S], F32, tag=f"xf{t}", name="xt")
                    for i in range(4):
                        nc.sync.dma_start(out=xt[:h, OFFS[i]:OFFS[i] + FEATS[i]],
                                          in_=xin[i].ap()[b, r0:r0 + h, :])
                    xct = pbf.tile([128, S], BF, tag=f"xc{t}", bufs=2, name="xct")
                    _layernorm4(nc, pln, eps_col, xt, h, xct)
                    xnqt = pbf.tile([128, S], BF, tag=f"xnq{t}", bufs=2, name="xnqt")
                    nc.vector.tensor_mul(out=xnqt[:h], in0=xct[:h], in1=wq_row[:h])
                    st["xf"].append(xt); st["xc"].append(xct); st["xnq"].append(xnqt)
                xcT = pT.tile([128, EC, N], BF, tag="xcT", bufs=2)
                for t, (r0, h) in enumerate(NT):
                    nc.sync.dma_start(out=xcT[:, :, r0:r0 + h], in_=st["xc"][t][:h, 0:S],
                                      transpose=True)
                st["xcT"] = xcT
                st["expatt"] = pe.tile([128, EC, S], BF, tag="expatt", name="expatt")
                return st

            def gexp_chunk(st, j):
                """gram-matrix matmuls + per-scale exp for e-chunk j"""
                xc, xnq, expatt = st["xc"], st["xnq"], st["expatt"]
                g4 = [ps.tile([128, 512], F32, tag="ps", name=f"g{q_}") for q_ in range(4)]
                for t, (r0, h) in enumerate(NT):
                    lhsT = xc[t][:h, 128 * j:128 * (j + 1)]
                    kw = dict(start=(t == 0), stop=(t == 1))
                    nc.tensor.matmul(g4[0][:, 0:384], lhsT, xnq[t][:h, 0:384], **kw)
                    nc.tensor.matmul(g4[1][:, 0:512], lhsT, xnq[t][:h, 384:896], **kw)
                    nc.tensor.matmul(g4[2][:, 0:512], lhsT, xnq[t][:h, 896:1408], **kw)
                    nc.tensor.matmul(g4[3][:, 0:512], lhsT, xnq[t][:h, 1408:1920], **kw)
                exp_src = [(0, 128, g4[0][:, 0:128]), (1, 256, g4[0][:, 128:384]),
                           (2, 512, g4[1][:, 0:512]), (3, 512, g4[2][:, 0:512]),
                           (3, 512, g4[3][:, 0:512])]
                col = 0
                for i, w, src in exp_src:
                    nc.scalar.activation(out=expatt[:, j, col:col + w], in_=src,
                                         func=AFT.Exp, scale=wk_col[:, i, j:j + 1])
                    col += w

            def chav_pass(st, pa):
                xf, xcT, expatt = st["xf"], st["xcT"], st["expatt"]
                op = [[ps.tile([128, 512], F32, tag="ps", name=f"avp{t_}{s_}")
                       for s_ in range(2)] for t_ in range(2)]
                # v tiles are per-scale; build lazily, then run both token tiles
                for (i, places) in pa["scales"]:
                    vch = pv.tile([128, EC, 194], BF, tag="vch", name="vch")
                    for j in range(EC):
                        nc.vector.tensor_scalar_mul(out=vch[:, j, 0:192], in0=xcT[:, j, :],
                                                    scalar1=wv_col[:, i, j:j + 1])
                    nc.vector.memset(vch[:, :, 192:193], 1.0)
                    for (slot, c0, w, d0) in places:
                        for t, (r0, h) in enumerate(NT):
                            ncol = slice(0, 128) if t == 0 else slice(128, 193)
                            mw = 128 if t == 0 else 65
                            for j in range(EC):
                                nc.tensor.matmul(op[t][slot][0:mw, c0:c0 + w],
                                                 vch[:, j, ncol], expatt[:, j, d0:d0 + w],
                                                 start=(j == 0), stop=(j == EC - 1))
                # Z -> zw = Z/wp ; broadcast ; reciprocal
                zw = pz.tile([1, 2, 512], BF, tag="zw", name="zw")
                rbs = prb.tile([128, 2, 512], BF, tag="rbs", name="rbs")
                for slot, (g0, w) in enumerate(pa["slots"]):
                    rbp = ps.tile([128, 512], F32, tag="ps", name=f"rbp{slot}")
                    nc.vector.tensor_mul(out=zw[0:1, slot, 0:w], in0=op[1][slot][64:65, 0:w],
                                         in1=invwp[0:1, g0:g0 + w])
                    nc.tensor.matmul(rbp[:, 0:w], ones_k1, zw[0:1, slot, 0:w])
                    with nc.allow_low_precision(reason="1/Z to bf16"):
                        nc.vector.reciprocal(out=rbs[:, slot, 0:w], in_=rbp[:, 0:w])
                # x += out_T * rbs
                for t, (r0, h) in enumerate(NT):
                    tmp = ptmp.tile([128, 2, 512], BF, tag="etmp", name="tmp")
                    for slot, (g0, w) in enumerate(pa["slots"]):
                        nc.vector.tensor_mul(out=tmp[:h, slot, 0:w], in0=op[t][slot][:h, 0:w],
                                             in1=rbs[:h, slot, 0:w])
                        nc.vector.tensor_add(out=xf[t][:h, g0:g0 + w], in0=xf[t][:h, g0:g0 + w],
                                             in1=tmp[:h, slot, 0:w])

            def sp_pieces(st, b):
                """spatial branch as a list of closures for interleaved emission"""
                xf = st["xf"]

                def p_ln():
                    st["xns"], st["xnsp"] = [], []
                    for t, (r0, h) in enumerate(NT):
                        xnst = pbf.tile([128, S], BF, tag=f"xns{t}", name="xnst")
                        _layernorm4(nc, pln, eps_col, xf[t], h, xnst)
                        xnspt = pbf.tile([128, 4, 512], BF, tag=f"xnsp{t}", name="xnspt")
                        for hh in range(4):
                            nc.vector.tensor_copy(out=xnspt[:h, hh, 0:480],
                                                  in_=xnst[:h, 480 * hh:480 * (hh + 1)])
                        nc.vector.memset(xnspt[:, :, 480:512], 0.0)
                        st["xns"].append(xnst); st["xnsp"].append(xnspt)
                    xnsT = pT.tile([128, 16, N], BF, tag="xnsT", name="xnsT")
                    for t, (r0, h) in enumerate(NT):
                        nc.sync.dma_start(out=xnsT[:, :, r0:r0 + h],
                                          in_=st["xnsp"][t][:h, :, :].rearrange("p a b -> p (a b)"),
                                          transpose=True)
                    st["xnsT"] = xnsT

                def p_scale(i):
                    o, c = OFFS[i], FEATS[i]
                    e = c // 4
                    xns, xnsT = st["xns"], st["xnsT"]
                    xnsu = pu.tile([128, 16, N], BF, tag="xnsu", bufs=2, name="xnsu")
                    for k in range(16):
                        nc.vector.tensor_scalar_mul(out=xnsu[:, k, :], in0=xnsT[:, k, :],
                                                    scalar1=u_col[:, i, k:k + 1])
                    expT = []
                    for t, (r0, mw) in enumerate(NT):
                        pl = [ps.tile([128, 512], F32, tag="ps", name=f"pl{q_}")
                              .rearrange("p (a b) -> p a b", a=2) for q_ in range(2)]
                        for hh in range(4):
                            for kc in range(4):
                                nc.tensor.matmul(pl[hh // 2][0:mw, hh % 2, 0:N],
                                                 xnsu[:, 4 * hh + kc, r0:r0 + mw],
                                                 xnsT[:, 4 * hh + kc, :],
                                                 start=(kc == 0), stop=(kc == 3))
                        eT = pse.tile([128, 4, N], BF, tag="expT", name="eT")
                        nc.scalar.activation(out=eT[:mw, 0:2, :], in_=pl[0][:mw, :, 0:N], func=AFT.Exp)
                        nc.scalar.activation(out=eT[:mw, 2:4, :], in_=pl[1][:mw, :, 0:N], func=AFT.Exp)
                        expT.append(eT)
                    vsp = []
                    for t, (r0, mw) in enumerate(NT):
                        vt = pvs.tile([128, 4, 258], BF, tag="vsp", name="vt")
                        nc.vector.tensor_mul(
                            out=vt[:mw, :, 0:e],
                            in0=xns[t][:mw, o:o + c].rearrange("p (hh e) -> p hh e", hh=4),
                            in1=wvp_row[:mw, o:o + c].rearrange("p (hh e) -> p hh e", hh=4))
                        nc.vector.memset(vt[:mw, :, e:e + 1], 1.0)
                        vsp.append(vt)
                    for t, (r0, nw) in enumerate(NT):
                        if e + 1 <= 256:  # 2 heads per psum bank
                            po = [ps.tile([128, 512], F32, tag="ps", name=f"po{q_}")
                                  .rearrange("p (a b) -> p a b", a=2) for q_ in range(2)]
                            heads = [po[hh // 2][:, hh % 2, :] for hh in range(4)]
                        else:             # scale 3: one head per bank
                            heads = [ps.tile([128, 512], F32, tag="ps", name=f"po4{q_}")
                                     for q_ in range(4)]
                        for hh in range(4):
                            for mt, (m0, mw) in enumerate(NT):
                                nc.tensor.matmul(heads[hh][0:nw, 0:e + 1],
                                                 expT[mt][:mw, hh, r0:r0 + nw],
                                                 vsp[mt][:mw, hh, 0:e + 1],
                                                 start=(mt == 0), stop=(mt == 1))
                        rz = prz.tile([128, 4], F32, tag="rz", name="rz")
                        if e + 1 <= 256:
                            for q in range(2):
                                nc.vector.reciprocal(
                                    out=rz[:nw, 2 * q:2 * q + 2],
                                    in_=po[q][:nw, :, e:e + 1].rearrange("p a o -> p (a o)"))
                        else:
                            for hh in range(4):
                                nc.vector.reciprocal(out=rz[:nw, hh:hh + 1],
                                                     in_=heads[hh][:nw, e:e + 1])
                        tsp = ptmp.tile([128, 2, 512], BF, tag="etmp", name="tsp")
                        tspf = tsp.rearrange("p a w -> p (a w)")
                        for hh in range(4):
                            nc.vector.tensor_scalar_mul(out=tspf[:nw, hh * e:(hh + 1) * e],
                                                        in0=heads[hh][:nw, 0:e],
                                                        scalar1=rz[:nw, hh:hh + 1])
                        nc.vector.tensor_add(out=xf[t][:nw, o:o + c], in0=xf[t][:nw, o:o + c],
                                             in1=tspf[:nw, 0:c])

                def p_store():
                    for t, (r0, h) in enumerate(NT):
                        for i in range(4):
                            nc.sync.dma_start(out=yout[i].ap()[b, r0:r0 + h, :],
                                              in_=xf[t][:h, OFFS[i]:OFFS[i] + FEATS[i]])

                return [p_ln] + [(lambda i=i: p_scale(i)) for i in range(4)] + [p_store]

            # ---- software-pipelined emission: spatial(b-1) between channel(b) parts
            pieces = []
            for b in range(bl):
                st = front(b)
                # spread the previous batch's spatial pieces through this
                # batch's G/exp chunk loop and AV passes
                emit_at = {1: 0, 4: 1, 7: 2, 10: 3, 13: 4}
                for j in range(EC):
                    gexp_chunk(st, j)
                    if j in emit_at and emit_at[j] < len(pieces):
                        pieces[emit_at[j]]()
                for k in range(5, len(pieces) - 1):
                    pieces[k]()
                chav_pass(st, CH_PASSES[0])
                if pieces:
                    pieces[-1]()  # store of b-1
                chav_pass(st, CH_PASSES[1])
                pieces = sp_pieces(st, b)
            for p in pieces:
                p()
    nc.compile()
    return nc


def kernel(x0, x1, x2, x3, params):
    xs = [np.ascontiguousarray(np.asarray(x, np.float32)) for x in (x0, x1, x2, x3)]
    assert xs[0].shape[0] % NCORES == 0
    bl = xs[0].shape[0] // NCORES
    par = _bld_params(params)
    nc = build(bl)
    in_maps = []
    for cid in range(NCORES):
        m = {f"x{i}": np.ascontiguousarray(xs[i][cid * bl:(cid + 1) * bl]) for i in range(4)}
        m.update(par)
        in_maps.append(m)
    res = None
    for attempt in range(3):
        try:
            res = run_bass_kernel_spmd(nc, in_maps, core_ids=list(range(NCORES)))
            break
        except Exception:
            # transient NRT_EXEC_UNIT_UNRECOVERABLE wedges have been observed
            # right after a previous heavy process; retry after clearing jax.
            if attempt == 2:
                raise
            import time
            try:
                import jax
                jax.clear_caches()
                getattr(jax, "clear_backends", lambda: None)()
            except Exception:
                pass
            time.sleep(5)
    outs = []
    for i in range(4):
        outs.append(np.concatenate([res.results[cid][f"y{i}"] for cid in range(NCORES)], axis=0))
    return tuple(outs)

